# revision 1
# baseline (speedup 1.0000x reference)
"""ChamferkNNDist kernel for Trainium2 (8 NeuronCores, pure data parallel).

Reference math (per batch element b, K=4096 points, 3 dims):
  chamfer_b = mean_i min_j ||adv_i - ori_j||^2
  dd_ij     = ||adv_i - adv_j||^2
  value_i   = mean of the 5 smallest dd_ij excluding self
  knn_b     = mean_i value_i * [value_i > mean(value) + 1.05*std(value, ddof=1)]
  loss      = 5 * mean_b chamfer_b + 3 * mean_b knn_b

Device strategy (one batch element per core):
  All pair terms are computed on the PE as u_ij = 2*a_i.b_j - ||b_j||^2 so that
  ||a_i - b_j||^2 = aa_i - u_ij  (aa_i per-row constant; preserves ordering).
  Inputs are fp32; to run the PE at bf16 rate each fp32 factor is split into
  three bf16 terms (hi/mid/lo) and the products are expanded into 21 bf16
  contraction rows -> fp32-accurate PSUM results at 1 column/cycle.
  chamfer: fused tensor_tensor_reduce (elementwise max of the two row halves +
  max-reduce) -> row max of u -> min distance.
  knn: DVE max8 per row half + merge -> top-8 u values (rank 1 = self),
  value_i = aa_i - mean(u ranks 2..6).
  Batch stats (mean/std/threshold/masked mean) on device via ones-matmul
  column sums; host only averages the 8 per-core (chamfer_b, knn_b) pairs.
"""

import os
import sys
from contextlib import ExitStack

import numpy as np

try:
    import concourse  # noqa: F401
except ImportError:  # staged repo location inside the container
    for _p in ("/opt/trn_rl_repo", os.path.expanduser("~/.axon_site/_ro/trn_rl_repo")):
        if os.path.isdir(_p):
            sys.path.insert(0, _p)
            break

import concourse.bacc as bacc
import concourse.tile as tile
from concourse import mybir

F32 = mybir.dt.float32
BF16 = mybir.dt.bfloat16
ALU = mybir.AluOpType
AX = mybir.AxisListType

NPTS = 4096
N_CORES = 8
K_NN = 5
ALPHA = 1.05
W_CHAMFER = 5.0
W_KNN = 3.0
NEG_INF = -3.0e38


def build_body(tc, ctx: ExitStack, adv, ori, out, npts, repeat=1,
               do_knn=True, do_cham=True, do_reduce=True,
               cham_mode="bt", pool_frac=0.0, pool_levels=3):
    """Emit the per-core program. adv/ori: [npts,3] f32 DRAM APs; out: [1,2] f32.

    repeat>1 re-runs the main chunk loop (identical results rewritten) — used
    only by timing harnesses to measure the steady-state loop rate via slope.
    """
    nc = tc.nc
    nch = npts // 128           # 128-row chunks of query points
    jh = npts // 2              # row half processed per PSUM tile
    mmn = min(512, jh)          # matmul moving free dim
    nmm = jh // mmn

    singles = ctx.enter_context(tc.tile_pool(name="singles", bufs=1))
    prep = ctx.enter_context(tc.tile_pool(name="prep", bufs=1))
    feat = ctx.enter_context(tc.tile_pool(name="feat", bufs=1))
    small = ctx.enter_context(tc.tile_pool(name="small", bufs=4))
    scrp = ctx.enter_context(tc.tile_pool(name="scrp", bufs=3))
    champ = ctx.enter_context(tc.tile_pool(name="champ", bufs=3))
    acc = ctx.enter_context(tc.tile_pool(name="acc", bufs=1))

    # ---------------- load points, point-major [128, nch, 3] ----------------
    P_adv = prep.tile([128, nch, 3], F32, tag="P_adv")
    nc.sync.dma_start(out=P_adv[:], in_=adv.rearrange("(c p) d -> p c d", p=128))
    P_ori = prep.tile([128, nch, 3], F32, tag="P_ori")
    nc.sync.dma_start(out=P_ori[:], in_=ori.rearrange("(c p) d -> p c d", p=128))

    # ---------------- squared norms ----------------
    sqa = prep.tile([128, nch, 3], F32, tag="sqa")
    nc.vector.tensor_mul(sqa[:], P_adv[:], P_adv[:])
    aa = acc.tile([128, nch, 1], F32, tag="aa")
    nc.vector.tensor_reduce(aa[:], sqa[:], axis=AX.X, op=ALU.add)
    sqo = prep.tile([128, nch, 3], F32, tag="sqo")
    nc.vector.tensor_mul(sqo[:], P_ori[:], P_ori[:])
    bb = acc.tile([128, nch, 1], F32, tag="bb")
    nc.vector.tensor_reduce(bb[:], sqo[:], axis=AX.X, op=ALU.add)

    # ---------------- bf16 3-term splits into staging tiles ----------------
    # Staging layout, point-major: S[:, c, k] = feature row k of chunk c.
    # Matmul row pairing (lhs from adv splits a=ah+am+al, rhs from 2b splits):
    #   k0-2 : ah * 2bh     k3-5  : am * 2bh    k6-8  : al * 2bh
    #   k9-11: ah * 2bm     k12-14: am * 2bm    k15-17: ah * 2bl
    #   k18-20: 1 * (-bb split terms)
    SLa = prep.tile([128, nch, 21], BF16, tag="SLa")
    SRa = prep.tile([128, nch, 21], BF16, tag="SRa")
    SRo = prep.tile([128, nch, 21], BF16, tag="SRo")

    def split3(dst_h, dst_m, dst_l, src, shape, tag, on_act=True):
        """dst_* bf16 APs <- hi/mid/lo bf16 split of f32 src (same shape)."""
        cast = nc.scalar.copy if on_act else (
            lambda o, i: nc.vector.tensor_copy(o, i))
        r1 = prep.tile(shape, F32, tag=f"{tag}_r1")
        r2 = prep.tile(shape, F32, tag=f"{tag}_r2")
        cast(dst_h, src)                           # hi = bf16(x)
        nc.vector.tensor_sub(r1[:], src, dst_h)    # f32 - bf16 (ALU is fp32)
        cast(dst_m, r1[:])                         # mid = bf16(x - hi)
        nc.vector.tensor_sub(r2[:], r1[:], dst_m)
        cast(dst_l, r2[:])                         # lo = bf16(x - hi - mid)

    sh3 = [128, nch, 3]
    sh1 = [128, nch, 1]


    def build_rhs(SR, P, nrm, tag):
        """rhs staging from point tile P and its norm tile nrm ([128,nch,1])."""
        p2 = prep.tile(sh3, F32, tag=f"{tag}_p2")
        nc.vector.tensor_scalar_mul(p2[:], P[:], 2.0)
        split3(SR[:, :, 0:3], SR[:, :, 9:12], SR[:, :, 15:18], p2[:], sh3, f"{tag}_c")
        nc.scalar.copy(SR[:, :, 3:6], SR[:, :, 0:3])
        nc.scalar.copy(SR[:, :, 6:9], SR[:, :, 0:3])
        nc.scalar.copy(SR[:, :, 12:15], SR[:, :, 9:12])
        nn = prep.tile(sh1, F32, tag=f"{tag}_nn")
        nc.vector.tensor_scalar_mul(nn[:], nrm[:], -1.0)
        split3(SR[:, :, 18:19], SR[:, :, 19:20], SR[:, :, 20:21], nn[:], sh1, f"{tag}_n")

    build_rhs(SRa, P_adv, aa, "sra")

    # lhs side (adv): ah/am/al + repeats + ones
    split3(SLa[:, :, 0:3], SLa[:, :, 3:6], SLa[:, :, 6:9], P_adv[:], sh3, "sla", on_act=False)
    nc.vector.tensor_copy(SLa[:, :, 9:12], SLa[:, :, 0:3])
    nc.vector.tensor_copy(SLa[:, :, 12:15], SLa[:, :, 3:6])
    nc.vector.tensor_copy(SLa[:, :, 15:18], SLa[:, :, 0:3])
    nc.gpsimd.memset(SLa[:, :, 18:21], 1.0)

    build_rhs(SRo, P_ori, bb, "sro")

    # ---------------- identity for PE transposes ----------------
    ident_i = singles.tile([128, 128], mybir.dt.int32, tag="identI")
    nc.gpsimd.iota(ident_i[:], pattern=[[1, 128]], base=0, channel_multiplier=-1)
    ident = singles.tile([128, 128], BF16, tag="ident")
    nc.vector.tensor_scalar(ident[:], ident_i[:], 0.0, None, op0=ALU.is_equal)

    # ---------------- transpose staging -> feature-major [21, npts] ----------------
    FAa = feat.tile([21, npts], BF16, tag="FAa")   # lhsT features (adv)
    FBa = feat.tile([21, npts], BF16, tag="FBa")   # rhs features (adv)
    FBo = feat.tile([21, npts], BF16, tag="FBo")   # rhs features (ori)
    cpb = min(8, nch)  # chunks per bf16 psum bank (8*128 = 1024 elems = 2KB)
    with tc.tile_pool(name="tpsum", bufs=2, space="PSUM") as tpsum:
        for S, F, on_act in ((SRa, FBa, False), (SLa, FAa, False), (SRo, FBo, True)):
            for c0 in range(0, nch, cpb):
                cn = min(cpb, nch - c0)
                pt = tpsum.tile([21, cpb * 128], BF16, tag="tp")
                for ci in range(cn):
                    nc.tensor.transpose(
                        pt[:, ci * 128:(ci + 1) * 128], S[:, c0 + ci, :], ident[:]
                    )
                if on_act:
                    nc.scalar.copy(
                        F[:, c0 * 128:(c0 + cn) * 128], pt[:, : cn * 128]
                    )
                else:
                    nc.vector.tensor_copy(
                        F[:, c0 * 128:(c0 + cn) * 128], pt[:, : cn * 128]
                    )

    # ---------------- main loop: per 128-row query chunk ----------------
    M0 = acc.tile([128, nch], F32, tag="M0")   # chamfer row max of u, half 0
    M1 = acc.tile([128, nch], F32, tag="M1")   # half 1
    S5 = acc.tile([128, nch], F32, tag="S5")   # sum of u ranks 2..6 (knn)
    Dm = acc.tile([128, nch], F32, tag="Dm")   # direct min distances (bt mode)
    nc.gpsimd.memset(M0[:], NEG_INF)
    nc.gpsimd.memset(M1[:], NEG_INF)
    nc.gpsimd.memset(S5[:], 0.0)
    nc.gpsimd.memset(Dm[:], 0.0)

    with tc.tile_pool(name="dist", bufs=2, space="PSUM") as dist:
        for c in [c for _ in range(repeat) for c in range(nch)]:
            lhsT = FAa[:, c * 128:(c + 1) * 128]

            u16 = small.tile([128, 16], F32, tag="u16")

            def knn_half(h):
                if not do_knn:
                    return
                ps = dist.tile([128, jh], F32, tag="dist")
                for q in range(nmm):
                    j0 = h * jh + q * mmn
                    nc.tensor.matmul(
                        ps[:, q * mmn:(q + 1) * mmn], lhsT, FBa[:, j0:j0 + mmn],
                        start=True, stop=True,
                    )
                if do_reduce:
                    nc.vector.max(out=u16[:, h * 8:(h + 1) * 8], in_=ps[:])

            cham_sb = [None, None]

            def cham_half(h):
                if not do_cham:
                    return
                ps = dist.tile([128, jh], F32, tag="dist")
                for q in range(nmm):
                    j0 = h * jh + q * mmn
                    nc.tensor.matmul(
                        ps[:, q * mmn:(q + 1) * mmn], lhsT, FBo[:, j0:j0 + mmn],
                        start=True, stop=True,
                    )
                if not do_reduce:
                    return
                if cham_mode == "ttr2":
                    # ACT drains the PSUM tile to SBUF; one fused DVE
                    # max+max-reduce per chunk handles both halves (below).
                    cb = champ.tile([128, jh], F32, tag=f"cb{h}")
                    nc.scalar.copy(cb[:], ps[:])
                    cham_sb[h] = cb
                    return
                if cham_mode == "bt":
                    # ACT drains PSUM with the distance transform d = aa - u
                    # (exact fp32 inside ACT), written as bf16; DVE then runs
                    # a 2x-mode pairwise-min tree per chunk (below).
                    db = champ.tile([128, jh], BF16, tag=f"db{h}")
                    nc.scalar.activation(
                        db[:], ps[:], mybir.ActivationFunctionType.Identity,
                        bias=aa[:, c, 0:1], scale=-1.0,
                    )
                    cham_sb[h] = db
                    return
                Mh = M0 if h == 0 else M1
                if cham_mode == "reduce":
                    nc.vector.tensor_reduce(
                        Mh[:, c:c + 1], ps[:], axis=AX.X, op=ALU.max
                    )
                elif cham_mode == "pool" and jh >= 512:
                    # ACT copies the tile out of PSUM; Pool runs the first
                    # pool_levels pairwise-max levels; DVE finishes with a
                    # fused elementwise-max + max-reduce on the remainder.
                    cb = champ.tile([128, jh], F32, tag="cb")
                    nc.scalar.copy(cb[:], ps[:])
                    cbuf = champ.tile([128, jh // 2], F32, tag="cbuf")
                    w = jh // 2
                    nc.gpsimd.tensor_tensor(
                        cbuf[:, :w], cb[:, :w], cb[:, w:2 * w], op=ALU.max
                    )
                    cur, oth = cbuf, cb
                    w //= 2
                    for _ in range(pool_levels - 1):
                        nc.gpsimd.tensor_tensor(
                            oth[:, :w], cur[:, :w], cur[:, w:2 * w], op=ALU.max
                        )
                        cur, oth = oth, cur
                        w //= 2
                    scr = scrp.tile([128, w], F32, tag="scr")
                    nc.vector.tensor_tensor_reduce(
                        out=scr[:], in0=cur[:, :w], in1=cur[:, w:2 * w],
                        scale=1.0, scalar=NEG_INF, op0=ALU.max, op1=ALU.max,
                        accum_out=Mh[:, c:c + 1],
                    )
                else:
                    # fused: elementwise max of the two tile halves (one
                    # operand staged to SBUF by ACT) + max-reduce, one pass
                    hw_ = jh // 2
                    sb1 = champ.tile([128, hw_], F32, tag="ch1")
                    nc.scalar.copy(sb1[:], ps[:, hw_:])
                    scr = scrp.tile([128, hw_], F32, tag="scr")
                    nc.vector.tensor_tensor_reduce(
                        out=scr[:], in0=ps[:, :hw_], in1=sb1[:],
                        scale=1.0, scalar=NEG_INF, op0=ALU.max, op1=ALU.max,
                        accum_out=Mh[:, c:c + 1],
                    )

            # interleave so DVE (knn) and ACT/Pool (chamfer) drain PSUM
            # concurrently from different tiles
            knn_half(0)
            cham_half(0)
            knn_half(1)
            cham_half(1)

            if do_cham and do_reduce and cham_mode == "ttr2":
                scr = scrp.tile([128, jh], F32, tag="scr")
                nc.vector.tensor_tensor_reduce(
                    out=scr[:], in0=cham_sb[0][:], in1=cham_sb[1][:],
                    scale=1.0, scalar=NEG_INF, op0=ALU.max, op1=ALU.max,
                    accum_out=M0[:, c:c + 1],
                )
            if do_cham and do_reduce and cham_mode == "bt":
                # bf16 pairwise-min tree at DVE 2x mode, then 1x reduce tail.
                bt1 = scrp.tile([128, jh], BF16, tag="bt1")
                nc.vector.tensor_tensor(
                    bt1[:], cham_sb[0][:], cham_sb[1][:], op=ALU.min
                )
                w = jh // 2
                bt2 = scrp.tile([128, w], BF16, tag="bt2")
                nc.vector.tensor_tensor(
                    bt2[:], bt1[:, :w], bt1[:, w:2 * w], op=ALU.min
                )
                w //= 2
                nc.vector.tensor_tensor(
                    bt1[:, :w], bt2[:, :w], bt2[:, w:2 * w], op=ALU.min
                )
                w //= 2
                nc.vector.tensor_tensor(
                    bt2[:, :w], bt1[:, :w], bt1[:, w:2 * w], op=ALU.min
                )
                nc.vector.tensor_reduce(
                    Dm[:, c:c + 1], bt2[:, :w], axis=AX.X, op=ALU.min
                )

            if do_knn and do_reduce:
                u8 = small.tile([128, 8], F32, tag="u8")
                nc.vector.max(out=u8[:], in_=u16[:])
                nc.vector.tensor_reduce(
                    S5[:, c:c + 1], u8[:, 1:6], axis=AX.X, op=ALU.add
                )

    # ---------------- finalize: per-batch scalars ----------------
    aa2 = aa[:, :, 0]
    ones = singles.tile([128, 1], F32, tag="ones")
    nc.vector.memset(ones[:], 1.0)

    D = acc.tile([128, nch], F32, tag="D")       # min chamfer distances
    if cham_mode == "bt":
        nc.vector.tensor_copy(D[:], Dm[:])
    else:
        M = acc.tile([128, nch], F32, tag="M")
        nc.vector.tensor_tensor(M[:], M0[:], M1[:], op=ALU.max)
        nc.vector.tensor_sub(D[:], aa2, M[:])
    Vt = acc.tile([128, nch], F32, tag="Vt")
    nc.vector.tensor_scalar_mul(Vt[:], S5[:], 1.0 / K_NN)
    VAL = acc.tile([128, nch], F32, tag="VAL")   # knn value_i
    nc.vector.tensor_sub(VAL[:], aa2, Vt[:])
    V2 = acc.tile([128, nch], F32, tag="V2")
    nc.vector.tensor_mul(V2[:], VAL[:], VAL[:])

    n = float(npts)
    st = small.tile([1, 12], F32, tag="st")
    outsb = small.tile([1, 2], F32, tag="outsb")
    with tc.tile_pool(name="cspsum", bufs=1, space="PSUM") as csp:
        cs = csp.tile([1, 3 * nch], F32, tag="cs")
        nc.tensor.matmul(cs[:, 0:nch], ones[:], D[:], start=True, stop=True)
        nc.tensor.matmul(cs[:, nch:2 * nch], ones[:], VAL[:], start=True, stop=True)
        nc.tensor.matmul(cs[:, 2 * nch:3 * nch], ones[:], V2[:], start=True, stop=True)
        nc.vector.tensor_reduce(st[:, 0:1], cs[:, 0:nch], axis=AX.X, op=ALU.add)
        nc.vector.tensor_reduce(st[:, 1:2], cs[:, nch:2 * nch], axis=AX.X, op=ALU.add)
        nc.vector.tensor_reduce(st[:, 2:3], cs[:, 2 * nch:3 * nch], axis=AX.X, op=ALU.add)

        # chamfer_b
        nc.vector.tensor_scalar_mul(outsb[:, 0:1], st[:, 0:1], 1.0 / n)
        # value stats: mean, var (ddof=1), threshold
        nc.vector.tensor_scalar_mul(st[:, 3:4], st[:, 1:2], 1.0 / n)          # mean
        nc.vector.tensor_mul(st[:, 4:5], st[:, 1:2], st[:, 1:2])              # sumV^2
        nc.vector.tensor_scalar_mul(st[:, 5:6], st[:, 4:5], 1.0 / n)
        nc.vector.tensor_sub(st[:, 6:7], st[:, 2:3], st[:, 5:6])
        nc.vector.tensor_scalar_mul(st[:, 7:8], st[:, 6:7], 1.0 / (n - 1.0))  # var
        nc.scalar.sqrt(st[:, 8:9], st[:, 7:8])                                # std
        nc.vector.tensor_scalar_mul(st[:, 9:10], st[:, 8:9], ALPHA)
        nc.vector.tensor_add(st[:, 10:11], st[:, 3:4], st[:, 9:10])           # thr

        thrb = small.tile([128, 1], F32, tag="thrb")
        nc.gpsimd.partition_broadcast(thrb[:], st[:, 10:11])
        G = acc.tile([128, nch], F32, tag="G")
        nc.vector.tensor_scalar(G[:], VAL[:], thrb[:, 0:1], None, op0=ALU.is_gt)
        VM = acc.tile([128, nch], F32, tag="VM")
        nc.vector.tensor_mul(VM[:], VAL[:], G[:])
        cs2 = csp.tile([1, nch], F32, tag="cs2")
        nc.tensor.matmul(cs2[:, 0:nch], ones[:], VM[:], start=True, stop=True)
        nc.vector.tensor_reduce(st[:, 11:12], cs2[:, 0:nch], axis=AX.X, op=ALU.add)
        nc.vector.tensor_scalar_mul(outsb[:, 1:2], st[:, 11:12], 1.0 / n)

    nc.sync.dma_start(out=out[0:1, 0:2], in_=outsb[:])


def build_nc(npts=NPTS):
    nc = bacc.Bacc("TRN2", target_bir_lowering=False, debug=False)
    adv = nc.dram_tensor("adv", [npts, 3], F32, kind="ExternalInput")
    ori = nc.dram_tensor("ori", [npts, 3], F32, kind="ExternalInput")
    out = nc.dram_tensor("out", [1, 2], F32, kind="ExternalOutput")
    with tile.TileContext(nc) as tc, ExitStack() as ctx:
        build_body(tc, ctx, adv.ap(), ori.ap(), out.ap(), npts)
    nc.compile()
    return nc


_NC_CACHE = {}


def _get_nc(npts=NPTS):
    if npts not in _NC_CACHE:
        _NC_CACHE[npts] = build_nc(npts)
    return _NC_CACHE[npts]


def kernel(**inputs) -> np.ndarray:
    from concourse.bass_utils import run_bass_kernel_spmd

    adv = np.ascontiguousarray(np.asarray(inputs["adv_pc"], dtype=np.float32))
    ori = np.ascontiguousarray(np.asarray(inputs["ori_pc"], dtype=np.float32))
    B = adv.shape[0]
    assert B == N_CORES and adv.shape[1] == NPTS, (adv.shape, ori.shape)

    nc = _get_nc()
    in_maps = [{"adv": adv[b], "ori": ori[b]} for b in range(B)]
    res = run_bass_kernel_spmd(nc, in_maps, core_ids=list(range(N_CORES)))
    parts = np.stack([r["out"][0] for r in res.results])  # [B, 2]
    loss = W_CHAMFER * parts[:, 0].mean() + W_KNN * parts[:, 1].mean()
    return np.float32(loss)



# revision 30
# speedup vs baseline: 1.0483x; 1.0483x over previous
"""ChamferkNNDist kernel for Trainium2 (8 NeuronCores, pure data parallel).

Reference math (per batch element b, K=4096 points, 3 dims):
  chamfer_b = mean_i min_j ||adv_i - ori_j||^2
  dd_ij     = ||adv_i - adv_j||^2
  value_i   = mean of the 5 smallest dd_ij excluding self
  knn_b     = mean_i value_i * [value_i > mean(value) + 1.05*std(value, ddof=1)]
  loss      = 5 * mean_b chamfer_b + 3 * mean_b knn_b

Device strategy (one batch element per core):
  The PE emits NEGATED squared distances directly: the 13-row bf16
  contraction computes -d_ij = 2 a_i.b_j - |b_j|^2 - |a_i|^2 with every
  fp32 factor compensated-split into bf16 hi+lo (dropped lo*lo cross terms
  leave ~1e-4 abs error; the row-constant |a_i|^2 rides along as two extra
  lhsT rows against all-ones rhs rows, so the cancellation happens in fp32
  PSUM). Row pairing (lhsT x rhs), with A = 2a:
    k0-2: Ah.bh   k3-5: Al.bh   k6-8: Ah.bl   k9,10: 1*(-bb hi,lo)
    k11,12: (-aa hi,lo)*1
  Streaming cost is 1 column/cycle regardless of the 13 rows.

  Per 128-query chunk, PSUM holds -d in f32 quarters and every engine
  consumes in parallel at ~5.5us/chunk:
  - kNN:  DVE max8 directly on each PSUM quarter (top-8 of -d; rank 1 =
    self at ~0), one [128,32] merge max8, value_i = -mean(ranks 2..6).
  - chamfer: ACT drains quarters to a bf16 [128,4096] row (relative
    precision is preserved because -d is small near the min); Pool (GPSIMD,
    SBUF-only) runs two contiguous-half max levels 4096 -> 1024; DVE
    finishes with one fused tensor_tensor_reduce (emitted two chunks late
    so DVE never waits on Pool). D = -max(-d).
  Chamfer matmuls trail the kNN matmuls by one chunk in the PE stream so
  Pool's pace never delays the quarters that feed DVE's max8.
  Batch stats (mean/std/threshold/masked mean) on device via ones-matmul
  column sums; host only averages the 8 per-core (chamfer_b, knn_b) pairs.
"""

import os
import sys
from contextlib import ExitStack

import numpy as np

try:
    import concourse  # noqa: F401
except ImportError:  # staged repo location inside the container
    for _p in ("/opt/trn_rl_repo", os.path.expanduser("~/.axon_site/_ro/trn_rl_repo")):
        if os.path.isdir(_p):
            sys.path.insert(0, _p)
            break

import concourse.bacc as bacc
import concourse.tile as tile
from concourse import mybir

F32 = mybir.dt.float32
BF16 = mybir.dt.bfloat16
ALU = mybir.AluOpType
AX = mybir.AxisListType

NPTS = 4096
N_CORES = 8
K_NN = 5
ALPHA = 1.05
W_CHAMFER = 5.0
W_KNN = 3.0
NEG_INF = -3.0e38
NROW = 13


def build_body(tc, ctx: ExitStack, adv, ori, out, npts):
    nc = tc.nc
    nch = npts // 128

    singles = ctx.enter_context(tc.tile_pool(name="singles", bufs=1))
    prep = ctx.enter_context(tc.tile_pool(name="prep", bufs=1))
    feat = ctx.enter_context(tc.tile_pool(name="feat", bufs=1))
    acc = ctx.enter_context(tc.tile_pool(name="acc", bufs=1))
    d16p = ctx.enter_context(tc.tile_pool(name="d16p", bufs=3))
    champ = ctx.enter_context(tc.tile_pool(name="champ", bufs=3))
    scrp = ctx.enter_context(tc.tile_pool(name="scrp", bufs=3))
    small = ctx.enter_context(tc.tile_pool(name="small", bufs=3))

    # ---------------- identity + PE warm-up ----------------
    # The PE clock ramps with sustained use; stream throwaway matmuls while
    # the DMA + staging prep runs so the transposes and first chunks start
    # at full speed.
    ident_i = singles.tile([128, 128], mybir.dt.int32, tag="identI")
    nc.gpsimd.iota(ident_i[:], pattern=[[1, 128]], base=0, channel_multiplier=-1)
    ident = singles.tile([128, 128], BF16, tag="ident")
    nc.vector.tensor_scalar(ident[:], ident_i[:], 0.0, None, op0=ALU.is_equal)
    wrm = singles.tile([128, 512], BF16, tag="wrm")
    nc.gpsimd.memset(wrm[:], 0.5)
    with tc.tile_pool(name="wpsum", bufs=1, space="PSUM") as wpsum:
        wps = wpsum.tile([128, 512], F32, tag="wps")
        for _ in range(12):
            nc.tensor.matmul(wps[:], wrm[:, 0:128], wrm[:], start=True, stop=True)

    # ---------------- load points (contiguous; point order is a
    # permutation, and every reduction here is permutation-invariant) ------
    P_a = prep.tile([128, nch, 3], F32, tag="P_a")
    nc.sync.dma_start(out=P_a[:], in_=adv.rearrange("(p c) d -> p c d", c=nch))
    P_o = prep.tile([128, nch, 3], F32, tag="P_o")
    nc.sync.dma_start(out=P_o[:], in_=ori.rearrange("(p c) d -> p c d", c=nch))

    # ---------------- negated squared norms ----------------
    def norms(P, tag):
        sq = prep.tile([128, nch, 3], F32, tag=f"sq{tag}")
        nc.vector.tensor_mul(sq[:], P[:], P[:])
        nn = prep.tile([128, nch, 1], F32, tag=f"nn{tag}")
        nc.vector.tensor_reduce(nn[:], sq[:], axis=AX.X, op=ALU.add)
        ng = prep.tile([128, nch, 1], F32, tag=f"ng{tag}")
        nc.vector.tensor_scalar_mul(ng[:], nn[:], -1.0)
        return nn, ng

    aa, naa = norms(P_a, "a")   # aa = |a|^2,  naa = -aa
    _bb, nbb = norms(P_o, "o")

    # naa bf16 hi/lo split, shared by S_L rows 11,12 and S_RA rows 9,10
    sh3 = [128, nch, 3]
    sh1 = [128, nch, 1]
    nah = prep.tile(sh1, BF16, tag="nah")
    nal = prep.tile(sh1, BF16, tag="nal")
    nc.scalar.copy(nah[:], naa[:])
    r0 = prep.tile(sh1, F32, tag="r0")
    nc.vector.tensor_sub(r0[:], naa[:], nah[:])
    nc.scalar.copy(nal[:], r0[:])

    # ---------------- bf16 hi/lo staging, point-major [128, nch, 13] ------
    S_L = prep.tile([128, nch, NROW], BF16, tag="S_L")
    # lhsT rows: Ah(0:3), Al(3:6), Ah dup(6:9), 1(9:11), nah(11), nal(12)
    B2 = prep.tile(sh3, F32, tag="B2")
    nc.vector.tensor_scalar_mul(B2[:], P_a[:], 2.0)
    nc.scalar.copy(S_L[:, :, 0:3], B2[:])                     # Ah = bf16(2a)
    rl = prep.tile(sh3, F32, tag="rl")
    nc.vector.tensor_sub(rl[:], B2[:], S_L[:, :, 0:3])
    nc.scalar.copy(S_L[:, :, 3:6], rl[:])                     # Al
    nc.vector.tensor_copy(S_L[:, :, 6:9], S_L[:, :, 0:3])
    nc.gpsimd.memset(S_L[:, :, 9:11], 1.0)
    nc.vector.tensor_copy(S_L[:, :, 11:12], nah[:])
    nc.vector.tensor_copy(S_L[:, :, 12:13], nal[:])

    def build_rhs(P, nh_src, nl_src, ng, tag, eng, cast):
        # rhs rows: bh(0:3), bh dup(3:6), bl(6:9), nb hi(9), nb lo(10),
        # ones(11:13). Chain on one engine so the two rhs builds overlap.
        S = prep.tile([128, nch, NROW], BF16, tag=f"S_{tag}")
        cast(S[:, :, 0:3], P[:])                              # bh
        r2 = prep.tile(sh3, F32, tag=f"r2_{tag}")
        eng.tensor_sub(r2[:], P[:], S[:, :, 0:3])
        cast(S[:, :, 6:9], r2[:])                             # bl
        eng.tensor_copy(S[:, :, 3:6], S[:, :, 0:3])
        if nh_src is not None:
            eng.tensor_copy(S[:, :, 9:10], nh_src[:])
            eng.tensor_copy(S[:, :, 10:11], nl_src[:])
        else:
            cast(S[:, :, 9:10], ng[:])                        # nb hi
            r3 = prep.tile(sh1, F32, tag=f"r3_{tag}")
            eng.tensor_sub(r3[:], ng[:], S[:, :, 9:10])
            cast(S[:, :, 10:11], r3[:])                       # nb lo
        nc.gpsimd.memset(S[:, :, 11:13], 1.0)
        return S

    S_RA = build_rhs(P_a, nah, nal, None, "ra", nc.gpsimd,
                     lambda o, i: nc.gpsimd.tensor_copy(o, i))
    S_RO = build_rhs(P_o, None, None, nbb, "ro", nc.vector,
                     lambda o, i: nc.scalar.copy(o, i))

    # ---------------- transpose staging -> feature-major [13, npts] -------
    T_L = feat.tile([NROW, npts], BF16, tag="T_L")
    T_RA = feat.tile([NROW, npts], BF16, tag="T_RA")
    T_RO = feat.tile([NROW, npts], BF16, tag="T_RO")

    S5 = acc.tile([128, nch], F32, tag="S5")   # sum of -d ranks 2..6 (knn)
    MU = acc.tile([128, nch], F32, tag="MU")   # chamfer row max of -d

    act_cp = lambda o, i: nc.scalar.copy(o, i)        # noqa: E731
    dve_cp = lambda o, i: nc.vector.tensor_copy(o, i)  # noqa: E731

    # knn: two max8 straight off the f32 -d PSUM halves (no drain).
    # cham: ACT drains halves to a bf16 [128,4096] row; one DVE ttr
    # (trailing two chunks) reduces it. Pool/GPSIMD cannot max on real HW,
    # and ttr/DMA cannot touch PSUM, so DVE carries all compare work.
    with tc.tile_pool(name="tpsum", bufs=2, space="PSUM") as tpsum, \
         tc.tile_pool(name="k0dist", bufs=1, space="PSUM") as k0dist:

        def tgroup(S, T, g, drain):
            pt = tpsum.tile([NROW, 1024], BF16, tag="pt")
            for ci in range(8):
                c = g * 8 + ci
                nc.tensor.transpose(
                    pt[:, ci * 128:(ci + 1) * 128], S[:, c, :], ident[:]
                )
            drain(T[:, g * 1024:(g + 1) * 1024], pt[:])

        for g in range(4):
            tgroup(S_L, T_L, g, dve_cp)
        for g in range(4):
            tgroup(S_RA, T_RA, g, act_cp if g < 2 else dve_cp)

        # chunk 0 kNN via a single-buffered tile so it overlaps the T_RO
        # transposes (the main dist pool needs all 8 banks)
        u16_0 = small.tile([128, 16], F32, tag="u16")
        lhsT0 = T_L[:, 0:128]
        for h in range(2):
            kh = k0dist.tile([128, 2048], F32, tag="k0")
            for q in range(4):
                j0 = h * 2048 + q * 512
                nc.tensor.matmul(kh[:, q * 512:(q + 1) * 512], lhsT0,
                                 T_RA[:, j0:j0 + 512], start=True, stop=True)
            nc.vector.max(out=u16_0[:, h * 8:(h + 1) * 8], in_=kh[:])
        for g in range(4):
            tgroup(S_RO, T_RO, g, act_cp)

    def dve_knn(c, u16):
        u8 = small.tile([128, 8], F32, tag="u8")
        nc.vector.max(out=u8[:], in_=u16[:])
        nc.vector.tensor_reduce(S5[:, c:c + 1], u8[:, 1:6], axis=AX.X,
                                op=ALU.add)

    def dve_ttr(c, D16c):
        # emitted two chunks late so DVE never stalls on ACT's drains.
        # tensor_tensor_reduce faults on real trn2, so this is a bf16
        # 2x-mode pairwise-max tree (the instruction mix the baseline
        # proved on hardware) plus one small reduce.
        t1 = scrp.tile([128, 2048], BF16, tag="t1")
        nc.vector.tensor_tensor(t1[:], D16c[:, 0:2048], D16c[:, 2048:4096],
                                op=ALU.max)
        t2 = scrp.tile([128, 1024], BF16, tag="t2")
        nc.vector.tensor_tensor(t2[:], t1[:, 0:1024], t1[:, 1024:2048],
                                op=ALU.max)
        t3 = scrp.tile([128, 512], BF16, tag="t3")
        nc.vector.tensor_tensor(t3[:], t2[:, 0:512], t2[:, 512:1024],
                                op=ALU.max)
        t4 = scrp.tile([128, 256], BF16, tag="t4")
        nc.vector.tensor_tensor(t4[:], t3[:, 0:256], t3[:, 256:512],
                                op=ALU.max)
        nc.vector.tensor_reduce(MU[:, c:c + 1], t4[:], axis=AX.X, op=ALU.max)

    with tc.tile_pool(name="dist", bufs=2, space="PSUM") as dist:

        def knn_half(c, h, u16):
            lhsT = T_L[:, c * 128:(c + 1) * 128]
            kh = dist.tile([128, 2048], F32, tag="d")
            for q in range(4):
                j0 = h * 2048 + q * 512
                nc.tensor.matmul(kh[:, q * 512:(q + 1) * 512], lhsT,
                                 T_RA[:, j0:j0 + 512], start=True, stop=True)
            nc.vector.max(out=u16[:, h * 8:(h + 1) * 8], in_=kh[:])

        def cham_half(c, h, D16c):
            lhsT = T_L[:, c * 128:(c + 1) * 128]
            ch = dist.tile([128, 2048], F32, tag="d")
            for q in range(4):
                j0 = h * 2048 + q * 512
                nc.tensor.matmul(ch[:, q * 512:(q + 1) * 512], lhsT,
                                 T_RO[:, j0:j0 + 512], start=True, stop=True)
            nc.scalar.copy(D16c[:, h * 2048:(h + 1) * 2048], ch[:])

        dve_knn(0, u16_0)
        D16s = {}
        prevD = d16p.tile([128, npts], BF16, tag="D16c")
        for c in range(1, nch):
            u16 = small.tile([128, 16], F32, tag="u16")
            knn_half(c, 0, u16)
            knn_half(c, 1, u16)
            cham_half(c - 1, 0, prevD)
            cham_half(c - 1, 1, prevD)
            D16s[c - 1] = prevD
            prevD = d16p.tile([128, npts], BF16, tag="D16c")
            dve_knn(c, u16)
            if c >= 2:
                dve_ttr(c - 2, D16s.pop(c - 2))
        cham_half(nch - 1, 0, prevD)
        cham_half(nch - 1, 1, prevD)
        dve_ttr(nch - 2, D16s.pop(nch - 2))
        dve_ttr(nch - 1, prevD)

    # ---------------- finalize: per-batch scalars ----------------
    ones = singles.tile([128, 1], F32, tag="ones")
    nc.vector.memset(ones[:], 1.0)

    D = acc.tile([128, nch], F32, tag="D")     # chamfer min distances
    nc.vector.tensor_scalar_mul(D[:], MU[:], -1.0)
    VAL = acc.tile([128, nch], F32, tag="VAL")  # knn value_i
    nc.vector.tensor_scalar_mul(VAL[:], S5[:], -1.0 / K_NN)
    V2 = acc.tile([128, nch], F32, tag="V2")
    nc.vector.tensor_mul(V2[:], VAL[:], VAL[:])

    n = float(npts)
    st = small.tile([1, 12], F32, tag="st")
    outsb = small.tile([1, 2], F32, tag="outsb")
    with tc.tile_pool(name="cspsum", bufs=1, space="PSUM") as csp:
        cs = csp.tile([1, 3 * nch], F32, tag="cs")
        nc.tensor.matmul(cs[:, 0:nch], ones[:], D[:], start=True, stop=True)
        nc.tensor.matmul(cs[:, nch:2 * nch], ones[:], VAL[:], start=True, stop=True)
        nc.tensor.matmul(cs[:, 2 * nch:3 * nch], ones[:], V2[:], start=True, stop=True)
        nc.vector.tensor_reduce(
            st[:, 0:3], cs[:].rearrange("o (g x) -> o g x", g=3),
            axis=AX.X, op=ALU.add,
        )

        # chamfer_b
        nc.vector.tensor_scalar_mul(outsb[:, 0:1], st[:, 0:1], 1.0 / n)
        # value stats: mean, var (ddof=1), threshold
        nc.vector.tensor_scalar_mul(st[:, 3:4], st[:, 1:2], 1.0 / n)          # mean
        nc.vector.tensor_mul(st[:, 4:5], st[:, 1:2], st[:, 1:2])              # sumV^2
        nc.vector.tensor_scalar_mul(st[:, 5:6], st[:, 4:5], 1.0 / n)
        nc.vector.tensor_sub(st[:, 6:7], st[:, 2:3], st[:, 5:6])
        nc.vector.tensor_scalar_mul(st[:, 7:8], st[:, 6:7], 1.0 / (n - 1.0))  # var
        nc.scalar.sqrt(st[:, 8:9], st[:, 7:8])                                # std
        nc.vector.tensor_scalar_mul(st[:, 9:10], st[:, 8:9], ALPHA)
        nc.vector.tensor_add(st[:, 10:11], st[:, 3:4], st[:, 9:10])           # thr

        thrb = small.tile([128, 1], F32, tag="thrb")
        nc.gpsimd.partition_broadcast(thrb[:], st[:, 10:11])
        G = acc.tile([128, nch], F32, tag="G")
        nc.vector.tensor_scalar(G[:], VAL[:], thrb[:, 0:1], None, op0=ALU.is_gt)
        VM = acc.tile([128, nch], F32, tag="VM")
        nc.vector.tensor_mul(VM[:], VAL[:], G[:])
        cs2 = csp.tile([1, nch], F32, tag="cs2")
        nc.tensor.matmul(cs2[:, 0:nch], ones[:], VM[:], start=True, stop=True)
        nc.vector.tensor_reduce(st[:, 11:12], cs2[:, 0:nch], axis=AX.X, op=ALU.add)
        nc.vector.tensor_scalar_mul(outsb[:, 1:2], st[:, 11:12], 1.0 / n)

    nc.sync.dma_start(out=out[0:1, 0:2], in_=outsb[:])


def build_nc(npts=NPTS):
    nc = bacc.Bacc("TRN2", target_bir_lowering=False, debug=False)
    adv = nc.dram_tensor("adv", [npts, 3], F32, kind="ExternalInput")
    ori = nc.dram_tensor("ori", [npts, 3], F32, kind="ExternalInput")
    out = nc.dram_tensor("out", [1, 2], F32, kind="ExternalOutput")
    with tile.TileContext(nc) as tc, ExitStack() as ctx:
        build_body(tc, ctx, adv.ap(), ori.ap(), out.ap(), npts)
    nc.compile()
    return nc


_NC_CACHE = {}


def _get_nc(npts=NPTS):
    if npts not in _NC_CACHE:
        _NC_CACHE[npts] = build_nc(npts)
    return _NC_CACHE[npts]


def kernel(**inputs) -> np.ndarray:
    from concourse.bass_utils import run_bass_kernel_spmd

    adv = np.ascontiguousarray(np.asarray(inputs["adv_pc"], dtype=np.float32))
    ori = np.ascontiguousarray(np.asarray(inputs["ori_pc"], dtype=np.float32))
    B = adv.shape[0]
    assert B == N_CORES and adv.shape[1] == NPTS, (adv.shape, ori.shape)

    nc = _get_nc()
    in_maps = [{"adv": adv[b], "ori": ori[b]} for b in range(B)]
    res = run_bass_kernel_spmd(nc, in_maps, core_ids=list(range(N_CORES)))
    parts = np.stack([r["out"][0] for r in res.results])  # [B, 2]
    loss = W_CHAMFER * parts[:, 0].mean() + W_KNN * parts[:, 1].mean()
    return np.float32(loss)
